# revision 49
# baseline (speedup 1.0000x reference)
"""Trainium2 Bass kernel for a two-window sparse causal self-attention block.

Model (B=2, T=2048, C=1024):
  - 8 "short" heads: d_qk=32,  window 256
  - 8 "long"  heads: d_qk=128, window 1024
  - value/output head dim 64, output projection C x C.

Sharding (8 cores): data-parallel over batch (2) x head-parallel over head
groups (4). Core c = 4*b + g handles batch b and heads {2g, 2g+1} of both the
short and long sets. Each core computes its 4 heads' attention plus the
corresponding 256 rows of Wproj, producing a partial [T, C] output (bf16);
the host sums the 4 partials per batch element in fp32.

V2 design notes (vs the fp32r baseline):
  - bf16 operands everywhere on the PE (fp32 PSUM accumulation): halves HBM
    traffic for x / out, enables FWL fast weight loads, and doubles DVE
    throughput for the mask multiplies. Correctness gate is 2e-2.
  - windowed score/AV matmuls: for key-tile kb only the 128-rounded valid
    query window [max(q0,128kb), min(q0+512, 128kb+win+128)) is computed,
    cutting streamed PE columns ~50% (short) / ~25% (long) and cutting exp
    and mask element counts the same way. The band-image mask zeroes the
    rounding slack.
  - score windows are packed into [128, <=1024] PSUM group tiles so one
    scalar-engine exp serves many windows (matmuls split at the 512-col PSUM
    bank boundary).
  - engine assignment: exp -> Scalar; band masks -> GpSimd; copies /
    reciprocal / normalization multiplies -> Vector; projections' PSUM
    drains -> Scalar (idle in stage A).
  - softmax normalization: ones-column in v accumulates query sums during
    AV; reciprocal_approx_fast (~18 bits) into r4 rows {0,32,64,96}; a K=33
    indicator matmul broadcasts two heads' reciprocal rows into a [128,512]
    tile per yts half. The normalization + output projection of query group
    g are deferred into group g+1's score phase so the PE never waits on the
    DVE chain.
  - x is DMA'd in 512-token chunks issued before the large weights so the
    first projection matmul starts early.
"""

import math

import numpy as np
from ml_dtypes import bfloat16, float8_e4m3

import concourse.bass as bass
import concourse.mybir as mybir
import concourse.tile as tile
from concourse.bass_utils import run_bass_kernel_spmd

F32 = mybir.dt.float32
BF16 = mybir.dt.bfloat16
FP8 = mybir.dt.float8e4
# fp8 weight pre-scale: lifts N(0, 0.02) weights out of the e4m3 subnormal
# range; compensated in the exp scale (q,k both carry x64) and in Wproj
# (x64 from Wv cancelled by 1/64 on wp host-side).
WSCALE = 64.0

B, T, C = 2, 2048, 1024
HS, DS = 8, 32
HL, DL = 8, 128
HD = 64
WIN_S, WIN_L = 256, 1024
NT = T // 128    # 16 t-blocks
NCB = C // 128   # 8 c-blocks
NG = T // 512    # 4 query groups
VW = HD + 1      # v columns + ones column for softmax sums
N_CORES = 8


def _split_waits(nc: bass.Bass) -> int:
    """Walrus in this env accepts at most 1 sync wait per instruction.
    Hoist extra waits onto same-engine InstNoOp instructions placed just
    before the owning instruction (same-engine program order preserves the
    blocking semantics)."""
    import bass_rust

    n_added = 0
    for f in nc.m.functions:
        for bb in f.blocks:
            insts = bb.instructions
            if not any(inst.sync_info and len(inst.sync_info.on_wait) > 1
                       for inst in insts):
                continue
            new = []
            for inst in insts:
                si = inst.sync_info
                waits = list(si.on_wait) if si else []
                if len(waits) > 1:
                    for i, w in enumerate(waits[:-1]):
                        nop = mybir.InstNoOp(
                            name=f"{inst.name}_hw{i}",
                            sync_info=bass_rust.SyncInfo(on_wait=[w], on_update=[]),
                            bass_nofuse=True,
                            engine=inst.engine,
                        )
                        new.append(nop)
                        n_added += 1
                    inst.sync_info = bass_rust.SyncInfo(
                        on_wait=waits[-1:], on_update=list(si.on_update))
                new.append(inst)
            bb.instructions = new
    return n_added


def _patch_tile_drain():
    """This walrus build rejects >1 sync wait on the TileContext tail drain
    ("Too many sync wait commands"). Re-emit the drain's waits as individual
    wait_ge instructions on the sync engine."""
    import bass_rust
    from concourse.tile import ScopedClock, TileContext

    def _drain_and_barrier(self, tick_clock, wait_clock):
        nc = self.nc
        drain_inst = nc.sync.drain()
        wait_clock.add_sem_waits(
            drain_inst.ins, ScopedClock({None: tick_clock.global_clock})
        )
        si = drain_inst.ins.sync_info
        waits = list(si.on_wait) if si is not None else []
        if len(waits) > 1:
            drain_inst.ins.sync_info = bass_rust.SyncInfo(on_wait=[], on_update=[])
            sems = {h.name: h for h in self.sems.allocated().values()}
            for w in waits:
                nc.sync.wait_ge(sems[w.ant_name], w.wait_value)
        nc.all_engine_barrier()
        popped = nc._tile_sem_poison_stack.pop()
        assert popped is self._sem_poison
        nc.clear_and_free_semaphores(list(self.sems.allocated().values()))
        nc.all_engine_barrier()

    TileContext._drain_and_barrier = _drain_and_barrier


_patch_tile_drain()


def _windows(q0: int, win: int):
    """Valid query windows per key-tile for queries [q0, q0+512).
    Returns [(kb, qlo, w, masked)]; qlo/w are 128-aligned and the window is
    the 128-rounded exact valid query range of key-tile kb."""
    kb_lo = max(0, q0 - win) // 128
    kb_hi = (q0 + 384) // 128
    out = []
    for kb in range(kb_lo, kb_hi + 1):
        qlo = max(q0, kb * 128)
        qhi = min(q0 + 512, kb * 128 + win + 128)
        w = qhi - qlo
        if w <= 0:
            continue
        masked = not (qlo >= kb * 128 + 128 and qlo + w <= kb * 128 + win)
        out.append((kb, qlo, w, masked))
    return out


def _groups(q0: int, win: int):
    """Pack windows into score-group tiles of <=1024 psum columns, masked
    windows first within each group so one band multiply covers the group's
    masked prefix. Returns [([(kb, qlo, w, masked, off), ...], mcols), ...]."""
    groups, cur, off = [], [], 0
    for kb, qlo, w, masked in _windows(q0, win):
        if off + w > 1024:
            groups.append(cur)
            cur, off = [], 0
        cur.append((kb, qlo, w, masked))
        off += w
    if cur:
        groups.append(cur)
    out = []
    for grp in groups:
        grp = sorted(grp, key=lambda t: not t[3])  # masked first, stable
        off, mcols, placed = 0, 0, []
        for kb, qlo, w, masked in grp:
            placed.append((kb, qlo, w, masked, off))
            if masked:
                mcols += w
            off += w
        out.append((placed, mcols))
    return out


def _gmask_layout():
    """Column layout of the concatenated masked-window band images, shared
    by host (image build) and device (slice offsets).
    Returns ({(q0, win): [goff per group]}, total_cols)."""
    offs, total = {}, 0
    for qg in range(NG):
        for win in (WIN_S, WIN_L):
            lst = []
            for placed, mcols in _groups(qg * 512, win):
                lst.append(total)
                total += mcols
            offs[(qg * 512, win)] = lst
    return offs, total


_GOFFS, _GMASK_COLS = _gmask_layout()


def _build_program() -> bass.Bass:
    nc = bass.Bass()

    # x pre-shuffled on host to [p, tch, cb, t] so each chunk DMA is one
    # contiguous segment per partition; weights pre-shuffled to [p, cb, d]
    # for the same reason. x and the qk projection weights are fp8e4 — the
    # qk projections run in DoubleRow mode (c-block pairs, 2x PE rate); the
    # error is washed out by softmax. The v projection stays bf16 (its
    # quantization error lands directly in the output and fails the gate),
    # so x is shipped in both dtypes.
    xt_d = nc.dram_tensor("xt", [128, NG * NCB * 512], FP8, kind="ExternalInput")
    xb_d = nc.dram_tensor("xb", [128, NG * NCB * 512], BF16, kind="ExternalInput")
    wsqk_d = nc.dram_tensor("wsqk", [128, NCB * 128], FP8, kind="ExternalInput")
    wql_d = nc.dram_tensor("wql", [128, NCB * 256], FP8, kind="ExternalInput")
    wkl_d = nc.dram_tensor("wkl", [128, NCB * 256], FP8, kind="ExternalInput")
    wv_d = nc.dram_tensor("wv", [128, NCB * 256], BF16, kind="ExternalInput")
    wp_d = nc.dram_tensor("wp", [256, C], BF16, kind="ExternalInput")
    gm_d = nc.dram_tensor("gmask", [128, _GMASK_COLS], BF16, kind="ExternalInput")
    ind_d = nc.dram_tensor("ind2", [97, 128], BF16, kind="ExternalInput")
    out_d = nc.dram_tensor("out", [T, C], BF16, kind="ExternalOutput")

    # q and k each carry a WSCALE factor from the fp8 weight pre-scale
    scale_s = 1.0 / math.sqrt(DS) / (WSCALE * WSCALE)
    scale_l = 1.0 / math.sqrt(DL) / (WSCALE * WSCALE)

    with nc.allow_low_precision(reason="bf16 attention pipeline, gate is 2e-2"), \
         tile.TileContext(nc) as tc:
        with (
            tc.tile_pool(name="const", bufs=1) as const,
            tc.tile_pool(name="qkp", bufs=1) as qkp,
            tc.tile_pool(name="vp", bufs=1) as vp,
            tc.tile_pool(name="scps", bufs=2, space="PSUM") as scps,
            tc.tile_pool(name="yhps", bufs=2, space="PSUM") as yhps,
            tc.tile_pool(name="pops", bufs=2, space="PSUM") as pops,
            tc.tile_pool(name="attnc", bufs=1) as attnc,
            tc.tile_pool(name="ptp", bufs=30) as ptp,
            tc.tile_pool(name="ytp", bufs=2) as ytp,
            tc.tile_pool(name="obp", bufs=3) as obp,
            tc.tile_pool(name="smallp", bufs=2) as smallp,
        ):
            # ---- persistent tiles ----
            qts = qkp.tile([64, T], BF16, tag="qts", name="qts")
            kts = qkp.tile([64, T], BF16, tag="kts", name="kts")
            qtl = [qkp.tile([128, T], BF16, tag=f"qtl{h}", name=f"qtl{h}") for h in range(2)]
            ktl = [qkp.tile([128, T], BF16, tag=f"ktl{h}", name=f"ktl{h}") for h in range(2)]
            # v tile [128 tokens, nt, head, VW]: all 4 heads interleaved so
            # the v-proj psum drains in one strided copy per token block
            vt = vp.tile([128, NT, 4, VW], BF16, tag="vt", name="vt")
            xt = qkp.tile([128, NG, NCB, 512], FP8, tag="xt", name="xt")
            xb = qkp.tile([128, NG, NCB, 512], BF16, tag="xb", name="xb")

            # ---- DMA issue order + queue spreading. dma_starts round-robin
            # the 16 hw queues; one queue moves ~22.5 B/ns, so a full x
            # chunk on a single queue is 23us (fp8) / 46us (bf16) — far too
            # slow for the software-pipelined schedule, which consumes chunk
            # t+1 during body t. Split every chunk across 4+ queues and
            # issue strictly in first-use order.
            xs_d = xt_d[:, :].rearrange("p (tch cb t) -> p tch cb t", tch=NG, cb=NCB)
            xbs_d = xb_d[:, :].rearrange("p (tch cb t) -> p tch cb t", tch=NG, cb=NCB)
            for cb in range(NCB):
                nc.sync.dma_start(xt[:, 0, cb], xs_d[:, 0, cb])
            wsqk = const.tile([128, NCB, 128], FP8, tag="wsqk", name="wsqk")
            nc.sync.dma_start(wsqk[:], wsqk_d[:, :].rearrange("p (cb d) -> p cb d", cb=NCB))
            gmask = attnc.tile([128, _GMASK_COLS], BF16, tag="gmask", name="gmask")
            nc.sync.dma_start(gmask[:, 0: _GMASK_COLS // 2], gm_d[:, 0: _GMASK_COLS // 2])
            nc.sync.dma_start(gmask[:, _GMASK_COLS // 2:], gm_d[:, _GMASK_COLS // 2:])
            wql = const.tile([128, NCB, 256], FP8, tag="wql", name="wql")
            nc.sync.dma_start(wql[:], wql_d[:, :].rearrange("p (cb d) -> p cb d", cb=NCB))
            wkl = const.tile([128, NCB, 256], FP8, tag="wkl", name="wkl")
            nc.sync.dma_start(wkl[:], wkl_d[:, :].rearrange("p (cb d) -> p cb d", cb=NCB))
            wv = const.tile([128, NCB, 256], BF16, tag="wv", name="wv")
            nc.sync.dma_start(wv[:], wv_d[:, :].rearrange("p (cb d) -> p cb d", cb=NCB))
            for j in range(4):
                nc.sync.dma_start(xb[:, 0, 2 * j: 2 * j + 2], xbs_d[:, 0, 2 * j: 2 * j + 2])
            for j in range(4):
                nc.sync.dma_start(xt[:, 1, 2 * j: 2 * j + 2], xs_d[:, 1, 2 * j: 2 * j + 2])
            for j in range(4):
                nc.sync.dma_start(xb[:, 1, 2 * j: 2 * j + 2], xbs_d[:, 1, 2 * j: 2 * j + 2])
            for tch in range(2, T // 512):
                for j in range(4):
                    nc.sync.dma_start(xt[:, tch, 2 * j: 2 * j + 2],
                                      xs_d[:, tch, 2 * j: 2 * j + 2])
                for j in range(4):
                    nc.sync.dma_start(xb[:, tch, 2 * j: 2 * j + 2],
                                      xbs_d[:, tch, 2 * j: 2 * j + 2])
            wp0 = attnc.tile([128, C], BF16, tag="wp0", name="wp0")
            nc.sync.dma_start(wp0[:, 0:512], wp_d[0:128, 0:512])
            nc.sync.dma_start(wp0[:, 512:1024], wp_d[0:128, 512:1024])
            wp1 = attnc.tile([128, C], BF16, tag="wp1", name="wp1")
            nc.sync.dma_start(wp1[:, 0:512], wp_d[128:256, 0:512])
            nc.sync.dma_start(wp1[:, 512:1024], wp_d[128:256, 512:1024])
            ind2 = attnc.tile([97, 128], BF16, tag="ind2", name="ind2")
            nc.sync.dma_start(ind2[:], ind_d[:, :])
            # rsum rows {0,32,64,96} collect per-head softmax sums; r4 holds
            # their reciprocals via ln/exp on the scalar engine (a DVE
            # reciprocal on [1,512] rows is ~6.5ns/elem — single-lane).
            # Other rows feed the K=33 indicator matmul as zero-weight
            # operands and must be finite.
            lnt = attnc.tile([97, 512], F32, tag="lnt", name="lnt")
            nc.vector.memset(lnt[:], 0.0)
            r4 = attnc.tile([97, 512], BF16, tag="r4", name="r4")
            nc.vector.memset(r4[:], 1.0)
            # ones columns of the v tile (strided view)
            nc.gpsimd.memset(vt[:, :, :, HD], 1.0)

            # PE warmup: ~4-6us of discarded matmuls during the startup DMA
            # wait releases the HAM clock throttle (4096-cycle activity
            # window) before the first real projection arrives
            wtile = attnc.tile([128, 640], BF16, tag="wtile", name="wtile")
            nc.gpsimd.memset(wtile[:], 0.0)
            for _ in range(14):
                wps = scps.tile([128, 1024], F32, tag="scps", name="scps")
                nc.tensor.matmul(wps[:, 0:512], wtile[:, 0:128], wtile[:, 128:640],
                                 start=True, stop=True)

            proj_jobs = [(wsqk, None, None)]
            for h in range(2):
                proj_jobs.append((wql, h, qtl[h]))
                proj_jobs.append((wkl, h, ktl[h]))

            def emit_proj_job(tch, ji):
                """One q/k projection job for token chunk tch (PE-dense
                filler between score groups; scalar drains). fp8 DoubleRow
                over c-block pairs."""
                w, h, dst = proj_jobs[ji]
                ps = pops.tile([128, 512], F32, tag="pops", name="pops")
                for j in range(NCB // 2):
                    cs = slice(2 * j, 2 * j + 2)
                    lhsT = w[:, cs, :] if h is None else w[:, cs, h * 128:(h + 1) * 128]
                    nc.tensor.matmul(
                        ps[:], lhsT, xt[:, tch, cs, :],
                        start=(j == 0), stop=(j == NCB // 2 - 1),
                        perf_mode=mybir.MatmulPerfMode.DoubleRow,
                    )
                sl = (slice(None), slice(tch * 512, (tch + 1) * 512))
                if dst is None:
                    # short-head q/k feed the first score groups right away:
                    # low-latency scalar drain
                    nc.scalar.copy(qts[sl], ps[0:64, :])
                    nc.scalar.copy(kts[sl], ps[64:128, :])
                else:
                    # long-head drains ride the DVE so they never delay the
                    # scalar exp stream (gpsimd cannot read PSUM)
                    nc.vector.tensor_copy(dst[sl], ps[:])

            def emit_vproj(tch, j):
                """One v-projection token block, bf16 (vector drains in one
                strided copy)."""
                tb = 4 * tch + j
                ps = pops.tile([128, 512], F32, tag="pops", name="pops")
                for cb in range(NCB):
                    nc.tensor.matmul(
                        ps[:, 0:256], xb[:, tch, cb, j * 128:(j + 1) * 128],
                        wv[:, cb, :],
                        start=(cb == 0), stop=(cb == NCB - 1),
                    )
                nc.vector.tensor_copy(
                    vt[:, tb, :, 0:HD],
                    ps[:, 0:256].rearrange("p (h d) -> p h d", h=4))

            def head_cfgs():
                cfgs = []
                for h in range(2):   # short heads
                    cfgs.append(dict(
                        kt=lambda kb, h=h: kts[32 * h: 32 * h + 32, kb * 128:(kb + 1) * 128],
                        qt=lambda qlo, w, h=h: qts[32 * h: 32 * h + 32, qlo: qlo + w],
                        vi=h, win=WIN_S, scale=scale_s,
                    ))
                for h in range(2):   # long heads
                    cfgs.append(dict(
                        kt=lambda kb, h=h: ktl[h][:, kb * 128:(kb + 1) * 128],
                        qt=lambda qlo, w, h=h: qtl[h][:, qlo: qlo + w],
                        vi=2 + h, win=WIN_L, scale=scale_l,
                    ))
                return cfgs

            cfgs = head_cfgs()

            def emit_score_group(qg, hi, gi, placed, mcols, pt_windows):
                """Score matmuls + exp + band mask for one score group;
                appends the pt windows to pt_windows."""
                cfg = cfgs[hi]
                q0 = qg * 512
                goffs = _GOFFS[(q0, cfg["win"])]
                if True:
                    used = placed[-1][4] + placed[-1][2]
                    st = scps.tile([128, 1024], F32, tag="scps", name="scps")
                    for kb, qlo, w, masked, off in placed:
                        # split at the 512-col psum bank boundary
                        cuts = [0]
                        if off < 512 < off + w:
                            cuts.append(512 - off)
                        cuts.append(w)
                        for a, b in zip(cuts, cuts[1:]):
                            nc.tensor.matmul(
                                st[:, off + a: off + b], cfg["kt"](kb),
                                cfg["qt"](qlo + a, b - a),
                                start=True, stop=True)
                    pt = ptp.tile([128, 1024], BF16, tag="pt", name="pt")
                    nc.scalar.activation(
                        pt[:, 0:used], st[:, 0:used],
                        mybir.ActivationFunctionType.Exp, scale=cfg["scale"])
                    if mcols:
                        # one multiply over the group's masked prefix; all on
                        # gpsimd (keeps the DVE free for psum drains) — the
                        # AV that consumes it runs a full body later, so mask
                        # latency is off the critical path
                        nc.gpsimd.tensor_tensor(
                            out=pt[:, 0:mcols], in0=pt[:, 0:mcols],
                            in1=gmask[:, goffs[gi]: goffs[gi] + mcols],
                            op=mybir.AluOpType.mult)
                    for kb, qlo, w, masked, off in placed:
                        pt_windows.append((kb, qlo, w, pt, off))

            def emit_av(qg, hi, pt_windows, yv2):
                """AV accumulation + per-head sums reciprocal + yv copy."""
                cfg = cfgs[hi]
                q0 = qg * 512
                yh = yhps.tile([VW, 512], F32, tag="yh", name="yh")
                n = len(pt_windows)
                for i, (kb, qlo, w, pt, off) in enumerate(pt_windows):
                    nc.tensor.matmul(
                        yh[:, qlo - q0: qlo - q0 + w],
                        vt[:, kb, cfg["vi"], :],
                        pt[:, off: off + w],
                        start=(i == 0), stop=(i == n - 1),
                        skip_group_check=True)
                # sums row drains on Scalar (Ln) while yv drains on DVE, so
                # the yh psum slot recycles at the speed of one copy, not two
                nc.scalar.activation(lnt[32 * hi: 32 * hi + 1, :], yh[HD: HD + 1, :],
                                     mybir.ActivationFunctionType.Ln)
                poff = 64 * (hi % 2)
                nc.vector.tensor_copy(yv2[hi // 2][poff: poff + 64, :], yh[0:HD, :])

            def emit_norm(yts, yv2, halves=(0, 1)):
                """Broadcast reciprocals via K=33 indicator matmuls and
                normalize into the bf16 yts tiles (mul reads rb psum)."""
                for half in halves:
                    rb = pops.tile([128, 512], F32, tag="pops", name="pops")
                    nc.tensor.matmul(rb[:], ind2[64 * half: 64 * half + 33, :],
                                     r4[64 * half: 64 * half + 33, :],
                                     start=True, stop=True)
                    nc.vector.tensor_mul(yts[half][:], yv2[half][:], rb[:])

            def emit_outproj(qg, yts, tail=False, subs=(0, 1, 2, 3)):
                q0 = qg * 512
                for sub in subs:
                    qs = q0 + sub * 128
                    ssl = (slice(None), slice(sub * 128, (sub + 1) * 128))
                    ob = obp.tile([128, 1024], BF16, tag="ob", name="ob")
                    for nh in range(2):
                        # po tiles live in the score-group pool: its slots are
                        # idle here, and pops stays free so the next body's
                        # projection jobs never wait on ob drains
                        po2 = scps.tile([128, 1024], F32, tag="scps", name="scps")
                        po = po2[:, 0:512]
                        nc.tensor.matmul(po[:], yts[0][ssl], wp0[:, nh * 512:(nh + 1) * 512],
                                         start=True, stop=False)
                        nc.tensor.matmul(po[:], yts[1][ssl], wp1[:, nh * 512:(nh + 1) * 512],
                                         start=False, stop=True)
                        osl = (slice(None), slice(nh * 512, (nh + 1) * 512))
                        # split psum drains between scalar and DVE
                        if nh == 1:
                            nc.vector.tensor_copy(ob[osl], po[:])
                        else:
                            nc.scalar.copy(ob[osl], po[:])
                    if tail:
                        # a [128,1024] out DMA is ~256KB on one queue (~11us);
                        # at the kernel tail nothing hides it — split across
                        # two queues
                        nc.sync.dma_start(out_d[qs: qs + 64, :], ob[0:64, :])
                        nc.sync.dma_start(out_d[qs + 64: qs + 128, :], ob[64:128, :])
                    else:
                        nc.sync.dma_start(out_d[qs: qs + 128, :], ob[:])

            # ---- merged pipeline, software-pipelined one group deep ----
            # Body t: chunk-t projections + group-t scores feed the scalar
            # exp stream, while group t-1's AV (whose exp/mask deps got a
            # full body of slack) and group t-2's output projection fill the
            # PE between score groups. The PE executes in order, so AV must
            # never sit at the end of the body that produced its deps — it
            # would bubble the PE on the exp+mask chain.
            av_pend = None    # (qg, all_pt, yv2, yts) awaiting AV in body qg+1
            norm_pend = None  # (qg, yts) normalized, awaiting outproj
            for t in range(NG):
                qg = t
                q0 = qg * 512
                yts = [ytp.tile([128, 512], BF16, tag=f"yts{i}", name=f"yts{i}")
                       for i in range(2)]
                yv2 = [smallp.tile([128, 512], BF16, tag=f"yv2{i}", name=f"yv2{i}")
                       for i in range(2)]

                # per-head score-group work queues
                sq = []
                for hi in range(4):
                    win = cfgs[hi]["win"]
                    sq.append([(hi, gi, placed, mcols)
                               for gi, (placed, mcols) in enumerate(_groups(q0, win))])
                all_pt = [[] for _ in range(4)]

                def pop_groups(heads, n):
                    done = 0
                    for hi in heads:
                        while sq[hi] and done < n:
                            _, gi, placed, mcols = sq[hi].pop(0)
                            emit_score_group(qg, hi, gi, placed, mcols, all_pt[hi])
                            done += 1
                        if done >= n:
                            break

                def pop_av(hi):
                    if av_pend is not None:
                        emit_av(av_pend[0], hi, av_pend[1][hi], av_pend[2])

                def pop_outproj(subs):
                    if norm_pend is not None:
                        emit_outproj(norm_pend[0], norm_pend[1], subs=subs)

                def emit_r4exp():
                    nc.scalar.activation(r4[:], lnt[:],
                                         mybir.ActivationFunctionType.Exp, scale=-1.0)

                if t == 0:
                    # body 0: dependency ladder (heads 0,1 need job 0; head
                    # 2 jobs 1,2; head 3 jobs 3,4)
                    emit_proj_job(0, 0)
                    pop_groups([0], 2)
                    emit_proj_job(0, 1)
                    pop_groups([0, 1], 2)
                    emit_proj_job(0, 2)
                    pop_groups([1, 2], 2)
                    emit_proj_job(0, 3)
                    pop_groups([2], 2)
                    emit_proj_job(0, 4)
                    pop_groups([2], 2)
                    emit_vproj(0, 0)
                    pop_groups([2, 3], 2)
                    emit_vproj(0, 1)
                    pop_groups([3], 2)
                    emit_vproj(0, 2)
                    pop_groups([3], 2)
                    emit_vproj(0, 3)
                    pop_groups([0, 1, 2, 3], 99)
                else:
                    # steady state: AV deps are a body old — spread AVs and
                    # outproj through the body as ready PE work
                    emit_proj_job(t, 0)
                    pop_av(0)
                    pop_groups([0], 2)
                    pop_outproj((0, 1))
                    emit_proj_job(t, 1)
                    pop_av(1)
                    pop_groups([0, 1], 2)
                    emit_proj_job(t, 2)
                    pop_groups([1, 2], 2)
                    pop_outproj((2, 3))
                    emit_proj_job(t, 3)
                    pop_av(2)
                    pop_groups([2], 2)
                    emit_proj_job(t, 4)
                    pop_groups([2], 2)
                    emit_vproj(t, 0)
                    pop_av(3)
                    emit_r4exp()
                    pop_groups([2, 3], 2)
                    emit_vproj(t, 1)
                    pop_groups([3], 2)
                    emit_vproj(t, 2)
                    pop_groups([3], 2)
                    emit_vproj(t, 3)
                    pop_groups([0, 1, 2, 3], 99)
                if av_pend is not None:
                    emit_norm(av_pend[3], av_pend[2])
                    norm_pend = (av_pend[0], av_pend[3])
                av_pend = (qg, all_pt, yv2, yts)

            # ---- tail: AV g3 with half-split normalization; outproj g2
            # interleaved as PE filler, outproj g3 last
            qg, all_pt, yv2, yts = av_pend
            pop_sub = norm_pend
            emit_av(qg, 0, all_pt[0], yv2)
            emit_outproj(pop_sub[0], pop_sub[1], subs=(0, 1), tail=True)
            emit_av(qg, 1, all_pt[1], yv2)
            nc.scalar.activation(r4[0:33, :], lnt[0:33, :],
                                 mybir.ActivationFunctionType.Exp, scale=-1.0)
            emit_av(qg, 2, all_pt[2], yv2)
            emit_outproj(pop_sub[0], pop_sub[1], subs=(2, 3), tail=True)
            emit_norm(yts, yv2, halves=(0,))
            emit_av(qg, 3, all_pt[3], yv2)
            nc.scalar.activation(r4[64:97, :], lnt[64:97, :],
                                 mybir.ActivationFunctionType.Exp, scale=-1.0)
            emit_norm(yts, yv2, halves=(1,))
            emit_outproj(qg, yts, tail=True)

    return nc


_PROGRAM = None


def _get_program() -> bass.Bass:
    global _PROGRAM
    if _PROGRAM is None:
        _PROGRAM = _build_program()
        _split_waits(_PROGRAM)
    return _PROGRAM


def _gmask_image() -> np.ndarray:
    """[128, _GMASK_COLS] 0/1 image: the masked-prefix windows of every score
    group, concatenated in _gmask_layout order. Window (kb, qlo, w) column u
    covers query qlo+u against key 128*kb+r."""
    img = np.zeros((128, _GMASK_COLS), dtype=np.float32)
    r = np.arange(128)[:, None]
    col = 0
    for qg in range(NG):
        for win in (WIN_S, WIN_L):
            for placed, mcols in _groups(qg * 512, win):
                for kb, qlo, w, masked, off in placed:
                    if not masked:
                        continue
                    u = np.arange(w)[None, :]
                    d = (qlo + u) - (kb * 128 + r)
                    img[:, col: col + w] = (d >= 0) & (d < win)
                    col += w
    assert col == _GMASK_COLS
    return img


def make_in_maps(x, Wqk_short, Wv_short, Wqk_long, Wv_long, Wproj):
    """Host-side sharding: per-core input dict for core c = 4*b + g."""
    x = np.asarray(x, dtype=np.float32)
    Wqk_short = np.asarray(Wqk_short, dtype=np.float32)
    Wv_short = np.asarray(Wv_short, dtype=np.float32)
    Wqk_long = np.asarray(Wqk_long, dtype=np.float32)
    Wv_long = np.asarray(Wv_long, dtype=np.float32)
    Wproj = np.asarray(Wproj, dtype=np.float32)
    assert x.shape == (B, T, C)

    bf = bfloat16
    f8 = float8_e4m3

    def shuf_w(w, dtype, scale=1.0):
        """[C, D] -> [128, NCB*D]: row p holds c-blocks cb at stride D."""
        d = w.shape[1]
        return np.ascontiguousarray(
            (w * scale).reshape(NCB, 128, d).transpose(1, 0, 2)
            .reshape(128, NCB * d)).astype(dtype)

    xts, xbs = [], []
    for b in range(B):
        # [C, T] -> [128, tch, cb, 512]
        xtb = x[b].T.reshape(NCB, 128, NG, 512).transpose(1, 2, 0, 3)
        xtb = np.ascontiguousarray(xtb.reshape(128, NG * NCB * 512))
        xts.append(xtb.astype(f8))
        xbs.append(xtb.astype(bf))
    gmask = _gmask_image().astype(bf)
    ind2 = np.zeros((97, 128), dtype=np.float32)  # cast to bf16 below
    ind2[0, 0:64] = 1.0
    ind2[32, 64:128] = 1.0
    ind2[64, 0:64] = 1.0
    ind2[96, 64:128] = 1.0
    in_maps = []
    for c in range(N_CORES):
        b, g = divmod(c, 4)
        wsqk = shuf_w(np.concatenate(
            [Wqk_short[:, g * 64:(g + 1) * 64],
             Wqk_short[:, 256 + g * 64: 256 + (g + 1) * 64]], axis=1), f8, WSCALE)
        wql = shuf_w(Wqk_long[:, g * 256:(g + 1) * 256], f8, WSCALE)
        wkl = shuf_w(Wqk_long[:, 1024 + g * 256: 1024 + (g + 1) * 256], f8, WSCALE)
        wv = shuf_w(np.concatenate(
            [Wv_short[:, g * 128:(g + 1) * 128],
             Wv_long[:, g * 128:(g + 1) * 128]], axis=1), bf)
        wp = np.ascontiguousarray(np.concatenate(
            [Wproj[g * 128:(g + 1) * 128, :],
             Wproj[512 + g * 128: 512 + (g + 1) * 128, :]], axis=0)).astype(bf)
        in_maps.append({
            "xt": xts[b], "xb": xbs[b], "wsqk": wsqk, "wql": wql, "wkl": wkl,
            "wv": wv, "wp": wp, "gmask": gmask, "ind2": ind2.astype(bf),
        })
    return in_maps


def gather(results) -> np.ndarray:
    out = np.empty((B, T, C), dtype=np.float32)
    for b in range(B):
        acc = np.zeros((T, C), dtype=np.float32)
        for g in range(4):
            acc += results[4 * b + g]["out"].astype(np.float32)
        out[b] = acc
    return out


def kernel(x, Wqk_short, Wv_short, Wqk_long, Wv_long, Wproj, **run_kwargs):
    nc = _get_program()
    in_maps = make_in_maps(x, Wqk_short, Wv_short, Wqk_long, Wv_long, Wproj)
    res = run_bass_kernel_spmd(nc, in_maps, core_ids=list(range(N_CORES)), **run_kwargs)
    out = gather(res.results)
    if run_kwargs:
        kernel.last_results = res
    return out



# revision 50
# speedup vs baseline: 1.0301x; 1.0301x over previous
"""Trainium2 Bass kernel for a two-window sparse causal self-attention block.

Model (B=2, T=2048, C=1024):
  - 8 "short" heads: d_qk=32,  window 256
  - 8 "long"  heads: d_qk=128, window 1024
  - value/output head dim 64, output projection C x C.

Sharding (8 cores): data-parallel over batch (2) x head-parallel over head
groups (4). Core c = 4*b + g handles batch b and heads {2g, 2g+1} of both the
short and long sets. Each core computes its 4 heads' attention plus the
corresponding 256 rows of Wproj, producing a partial [T, C] output (bf16);
the host sums the 4 partials per batch element in fp32.

V2 design notes (vs the fp32r baseline):
  - bf16 operands everywhere on the PE (fp32 PSUM accumulation): halves HBM
    traffic for x / out, enables FWL fast weight loads, and doubles DVE
    throughput for the mask multiplies. Correctness gate is 2e-2.
  - windowed score/AV matmuls: for key-tile kb only the 128-rounded valid
    query window [max(q0,128kb), min(q0+512, 128kb+win+128)) is computed,
    cutting streamed PE columns ~50% (short) / ~25% (long) and cutting exp
    and mask element counts the same way. The band-image mask zeroes the
    rounding slack.
  - score windows are packed into [128, <=1024] PSUM group tiles so one
    scalar-engine exp serves many windows (matmuls split at the 512-col PSUM
    bank boundary).
  - engine assignment: exp -> Scalar; band masks -> GpSimd; copies /
    reciprocal / normalization multiplies -> Vector; projections' PSUM
    drains -> Scalar (idle in stage A).
  - softmax normalization: ones-column in v accumulates query sums during
    AV; reciprocal_approx_fast (~18 bits) into r4 rows {0,32,64,96}; a K=33
    indicator matmul broadcasts two heads' reciprocal rows into a [128,512]
    tile per yts half. The normalization + output projection of query group
    g are deferred into group g+1's score phase so the PE never waits on the
    DVE chain.
  - x is DMA'd in 512-token chunks issued before the large weights so the
    first projection matmul starts early.
"""

import math

import numpy as np
from ml_dtypes import bfloat16, float8_e4m3

import concourse.bass as bass
import concourse.mybir as mybir
import concourse.tile as tile
from concourse.bass_utils import run_bass_kernel_spmd

F32 = mybir.dt.float32
BF16 = mybir.dt.bfloat16
FP8 = mybir.dt.float8e4
# fp8 weight pre-scale: lifts N(0, 0.02) weights out of the e4m3 subnormal
# range; compensated in the exp scale (q,k both carry x64) and in Wproj
# (x64 from Wv cancelled by 1/64 on wp host-side).
WSCALE = 64.0

B, T, C = 2, 2048, 1024
HS, DS = 8, 32
HL, DL = 8, 128
HD = 64
WIN_S, WIN_L = 256, 1024
NT = T // 128    # 16 t-blocks
NCB = C // 128   # 8 c-blocks
NG = T // 512    # 4 query groups
VW = HD + 1      # v columns + ones column for softmax sums
N_CORES = 8


def _split_waits(nc: bass.Bass) -> int:
    """Walrus in this env accepts at most 1 sync wait per instruction.
    Hoist extra waits onto same-engine InstNoOp instructions placed just
    before the owning instruction (same-engine program order preserves the
    blocking semantics)."""
    import bass_rust

    n_added = 0
    for f in nc.m.functions:
        for bb in f.blocks:
            insts = bb.instructions
            if not any(inst.sync_info and len(inst.sync_info.on_wait) > 1
                       for inst in insts):
                continue
            new = []
            for inst in insts:
                si = inst.sync_info
                waits = list(si.on_wait) if si else []
                if len(waits) > 1:
                    for i, w in enumerate(waits[:-1]):
                        nop = mybir.InstNoOp(
                            name=f"{inst.name}_hw{i}",
                            sync_info=bass_rust.SyncInfo(on_wait=[w], on_update=[]),
                            bass_nofuse=True,
                            engine=inst.engine,
                        )
                        new.append(nop)
                        n_added += 1
                    inst.sync_info = bass_rust.SyncInfo(
                        on_wait=waits[-1:], on_update=list(si.on_update))
                new.append(inst)
            bb.instructions = new
    return n_added


def _patch_tile_drain():
    """This walrus build rejects >1 sync wait on the TileContext tail drain
    ("Too many sync wait commands"). Re-emit the drain's waits as individual
    wait_ge instructions on the sync engine."""
    import bass_rust
    from concourse.tile import ScopedClock, TileContext

    def _drain_and_barrier(self, tick_clock, wait_clock):
        nc = self.nc
        drain_inst = nc.sync.drain()
        wait_clock.add_sem_waits(
            drain_inst.ins, ScopedClock({None: tick_clock.global_clock})
        )
        si = drain_inst.ins.sync_info
        waits = list(si.on_wait) if si is not None else []
        if len(waits) > 1:
            drain_inst.ins.sync_info = bass_rust.SyncInfo(on_wait=[], on_update=[])
            sems = {h.name: h for h in self.sems.allocated().values()}
            for w in waits:
                nc.sync.wait_ge(sems[w.ant_name], w.wait_value)
        nc.all_engine_barrier()
        popped = nc._tile_sem_poison_stack.pop()
        assert popped is self._sem_poison
        nc.clear_and_free_semaphores(list(self.sems.allocated().values()))
        nc.all_engine_barrier()

    TileContext._drain_and_barrier = _drain_and_barrier


_patch_tile_drain()


def _windows(q0: int, win: int):
    """Valid query windows per key-tile for queries [q0, q0+512).
    Returns [(kb, qlo, w, masked)]; qlo/w are 128-aligned and the window is
    the 128-rounded exact valid query range of key-tile kb."""
    kb_lo = max(0, q0 - win) // 128
    kb_hi = (q0 + 384) // 128
    out = []
    for kb in range(kb_lo, kb_hi + 1):
        qlo = max(q0, kb * 128)
        qhi = min(q0 + 512, kb * 128 + win + 128)
        w = qhi - qlo
        if w <= 0:
            continue
        masked = not (qlo >= kb * 128 + 128 and qlo + w <= kb * 128 + win)
        out.append((kb, qlo, w, masked))
    return out


def _groups(q0: int, win: int):
    """Pack windows into score-group tiles of <=1024 psum columns, masked
    windows first within each group so one band multiply covers the group's
    masked prefix. Returns [([(kb, qlo, w, masked, off), ...], mcols), ...]."""
    groups, cur, off = [], [], 0
    for kb, qlo, w, masked in _windows(q0, win):
        if off + w > 1024:
            groups.append(cur)
            cur, off = [], 0
        cur.append((kb, qlo, w, masked))
        off += w
    if cur:
        groups.append(cur)
    out = []
    for grp in groups:
        grp = sorted(grp, key=lambda t: not t[3])  # masked first, stable
        off, mcols, placed = 0, 0, []
        for kb, qlo, w, masked in grp:
            placed.append((kb, qlo, w, masked, off))
            if masked:
                mcols += w
            off += w
        out.append((placed, mcols))
    return out


def _gmask_layout():
    """Column layout of the concatenated masked-window band images, shared
    by host (image build) and device (slice offsets).
    Returns ({(q0, win): [goff per group]}, total_cols)."""
    offs, total = {}, 0
    for qg in range(NG):
        for win in (WIN_S, WIN_L):
            lst = []
            for placed, mcols in _groups(qg * 512, win):
                lst.append(total)
                total += mcols
            offs[(qg * 512, win)] = lst
    return offs, total


_GOFFS, _GMASK_COLS = _gmask_layout()


def _build_program() -> bass.Bass:
    nc = bass.Bass()

    # x pre-shuffled on host to [p, tch, cb, t] so each chunk DMA is one
    # contiguous segment per partition; weights pre-shuffled to [p, cb, d]
    # for the same reason. x and the qk projection weights are fp8e4 — the
    # qk projections run in DoubleRow mode (c-block pairs, 2x PE rate); the
    # error is washed out by softmax. The v projection stays bf16 (its
    # quantization error lands directly in the output and fails the gate),
    # so x is shipped in both dtypes.
    xt_d = nc.dram_tensor("xt", [128, NG * NCB * 512], FP8, kind="ExternalInput")
    xb_d = nc.dram_tensor("xb", [128, NG * NCB * 512], BF16, kind="ExternalInput")
    wsqk_d = nc.dram_tensor("wsqk", [128, NCB * 128], FP8, kind="ExternalInput")
    wql_d = nc.dram_tensor("wql", [128, NCB * 256], FP8, kind="ExternalInput")
    wkl_d = nc.dram_tensor("wkl", [128, NCB * 256], FP8, kind="ExternalInput")
    wv_d = nc.dram_tensor("wv", [128, NCB * 256], BF16, kind="ExternalInput")
    wp_d = nc.dram_tensor("wp", [256, C], BF16, kind="ExternalInput")
    gm_d = nc.dram_tensor("gmask", [128, _GMASK_COLS], BF16, kind="ExternalInput")
    ind_d = nc.dram_tensor("ind2", [97, 128], BF16, kind="ExternalInput")
    out_d = nc.dram_tensor("out", [T, C], BF16, kind="ExternalOutput")

    # q and k each carry a WSCALE factor from the fp8 weight pre-scale
    scale_s = 1.0 / math.sqrt(DS) / (WSCALE * WSCALE)
    scale_l = 1.0 / math.sqrt(DL) / (WSCALE * WSCALE)

    with nc.allow_low_precision(reason="bf16 attention pipeline, gate is 2e-2"), \
         tile.TileContext(nc) as tc:
        with (
            tc.tile_pool(name="const", bufs=1) as const,
            tc.tile_pool(name="qkp", bufs=1) as qkp,
            tc.tile_pool(name="vp", bufs=1) as vp,
            tc.tile_pool(name="scps", bufs=2, space="PSUM") as scps,
            tc.tile_pool(name="yhps", bufs=2, space="PSUM") as yhps,
            tc.tile_pool(name="pops", bufs=2, space="PSUM") as pops,
            tc.tile_pool(name="attnc", bufs=1) as attnc,
            tc.tile_pool(name="ptp", bufs=30) as ptp,
            tc.tile_pool(name="ytp", bufs=2) as ytp,
            tc.tile_pool(name="obp", bufs=3) as obp,
            tc.tile_pool(name="smallp", bufs=2) as smallp,
        ):
            # ---- persistent tiles ----
            qts = qkp.tile([64, T], BF16, tag="qts", name="qts")
            kts = qkp.tile([64, T], BF16, tag="kts", name="kts")
            qtl = [qkp.tile([128, T], BF16, tag=f"qtl{h}", name=f"qtl{h}") for h in range(2)]
            ktl = [qkp.tile([128, T], BF16, tag=f"ktl{h}", name=f"ktl{h}") for h in range(2)]
            # v tile [128 tokens, nt, head, VW]: all 4 heads interleaved so
            # the v-proj psum drains in one strided copy per token block
            vt = vp.tile([128, NT, 4, VW], BF16, tag="vt", name="vt")
            xt = qkp.tile([128, NG, NCB, 512], FP8, tag="xt", name="xt")
            xb = qkp.tile([128, NG, NCB, 512], BF16, tag="xb", name="xb")

            # ---- DMA issue order + queue spreading. dma_starts round-robin
            # the 16 hw queues; one queue moves ~22.5 B/ns, so a full x
            # chunk on a single queue is 23us (fp8) / 46us (bf16) — far too
            # slow for the software-pipelined schedule, which consumes chunk
            # t+1 during body t. Split every chunk across 4+ queues and
            # issue strictly in first-use order.
            xs_d = xt_d[:, :].rearrange("p (tch cb t) -> p tch cb t", tch=NG, cb=NCB)
            xbs_d = xb_d[:, :].rearrange("p (tch cb t) -> p tch cb t", tch=NG, cb=NCB)
            for cb in range(NCB):
                nc.sync.dma_start(xt[:, 0, cb], xs_d[:, 0, cb])
            wsqk = const.tile([128, NCB, 128], FP8, tag="wsqk", name="wsqk")
            nc.sync.dma_start(wsqk[:], wsqk_d[:, :].rearrange("p (cb d) -> p cb d", cb=NCB))
            gmask = attnc.tile([128, _GMASK_COLS], BF16, tag="gmask", name="gmask")
            nc.sync.dma_start(gmask[:, 0: _GMASK_COLS // 2], gm_d[:, 0: _GMASK_COLS // 2])
            nc.sync.dma_start(gmask[:, _GMASK_COLS // 2:], gm_d[:, _GMASK_COLS // 2:])
            wql = const.tile([128, NCB, 256], FP8, tag="wql", name="wql")
            nc.sync.dma_start(wql[:], wql_d[:, :].rearrange("p (cb d) -> p cb d", cb=NCB))
            wkl = const.tile([128, NCB, 256], FP8, tag="wkl", name="wkl")
            nc.sync.dma_start(wkl[:], wkl_d[:, :].rearrange("p (cb d) -> p cb d", cb=NCB))
            wv = const.tile([128, NCB, 256], BF16, tag="wv", name="wv")
            nc.sync.dma_start(wv[:], wv_d[:, :].rearrange("p (cb d) -> p cb d", cb=NCB))
            for j in range(4):
                nc.sync.dma_start(xb[:, 0, 2 * j: 2 * j + 2], xbs_d[:, 0, 2 * j: 2 * j + 2])
            for j in range(4):
                nc.sync.dma_start(xt[:, 1, 2 * j: 2 * j + 2], xs_d[:, 1, 2 * j: 2 * j + 2])
            for j in range(4):
                nc.sync.dma_start(xb[:, 1, 2 * j: 2 * j + 2], xbs_d[:, 1, 2 * j: 2 * j + 2])
            for tch in range(2, T // 512):
                for j in range(4):
                    nc.sync.dma_start(xt[:, tch, 2 * j: 2 * j + 2],
                                      xs_d[:, tch, 2 * j: 2 * j + 2])
                for j in range(4):
                    nc.sync.dma_start(xb[:, tch, 2 * j: 2 * j + 2],
                                      xbs_d[:, tch, 2 * j: 2 * j + 2])
            wp0 = attnc.tile([128, C], BF16, tag="wp0", name="wp0")
            nc.sync.dma_start(wp0[:, 0:512], wp_d[0:128, 0:512])
            nc.sync.dma_start(wp0[:, 512:1024], wp_d[0:128, 512:1024])
            wp1 = attnc.tile([128, C], BF16, tag="wp1", name="wp1")
            nc.sync.dma_start(wp1[:, 0:512], wp_d[128:256, 0:512])
            nc.sync.dma_start(wp1[:, 512:1024], wp_d[128:256, 512:1024])
            ind2 = attnc.tile([97, 128], BF16, tag="ind2", name="ind2")
            nc.sync.dma_start(ind2[:], ind_d[:, :])
            # rsum rows {0,32,64,96} collect per-head softmax sums; r4 holds
            # their reciprocals via ln/exp on the scalar engine (a DVE
            # reciprocal on [1,512] rows is ~6.5ns/elem — single-lane).
            # Other rows feed the K=33 indicator matmul as zero-weight
            # operands and must be finite.
            lnt = attnc.tile([97, 512], F32, tag="lnt", name="lnt")
            nc.vector.memset(lnt[:], 0.0)
            r4 = attnc.tile([97, 512], BF16, tag="r4", name="r4")
            nc.vector.memset(r4[:], 1.0)
            # ones columns of the v tile (strided view)
            nc.gpsimd.memset(vt[:, :, :, HD], 1.0)

            # PE warmup: ~4-6us of discarded matmuls during the startup DMA
            # wait releases the HAM clock throttle (4096-cycle activity
            # window) before the first real projection arrives
            wtile = attnc.tile([128, 640], BF16, tag="wtile", name="wtile")
            nc.gpsimd.memset(wtile[:], 0.0)
            for _ in range(14):
                wps = scps.tile([128, 1024], F32, tag="scps", name="scps")
                nc.tensor.matmul(wps[:, 0:512], wtile[:, 0:128], wtile[:, 128:640],
                                 start=True, stop=True)

            proj_jobs = [(wsqk, None, None)]
            for h in range(2):
                proj_jobs.append((wql, h, qtl[h]))
                proj_jobs.append((wkl, h, ktl[h]))

            def emit_proj_job(tch, ji):
                """One q/k projection job for token chunk tch (PE-dense
                filler between score groups; scalar drains). fp8 DoubleRow
                over c-block pairs."""
                w, h, dst = proj_jobs[ji]
                ps = pops.tile([128, 512], F32, tag="pops", name="pops")
                for j in range(NCB // 2):
                    cs = slice(2 * j, 2 * j + 2)
                    lhsT = w[:, cs, :] if h is None else w[:, cs, h * 128:(h + 1) * 128]
                    nc.tensor.matmul(
                        ps[:], lhsT, xt[:, tch, cs, :],
                        start=(j == 0), stop=(j == NCB // 2 - 1),
                        perf_mode=mybir.MatmulPerfMode.DoubleRow,
                    )
                sl = (slice(None), slice(tch * 512, (tch + 1) * 512))
                if dst is None:
                    # short-head q/k feed the first score groups right away:
                    # low-latency scalar drain
                    nc.scalar.copy(qts[sl], ps[0:64, :])
                    nc.scalar.copy(kts[sl], ps[64:128, :])
                else:
                    # long-head drains ride the DVE so they never delay the
                    # scalar exp stream (gpsimd cannot read PSUM)
                    nc.vector.tensor_copy(dst[sl], ps[:])

            def emit_vproj(tch, j):
                """One v-projection token block, bf16 (vector drains in one
                strided copy)."""
                tb = 4 * tch + j
                ps = pops.tile([128, 512], F32, tag="pops", name="pops")
                for cb in range(NCB):
                    nc.tensor.matmul(
                        ps[:, 0:256], xb[:, tch, cb, j * 128:(j + 1) * 128],
                        wv[:, cb, :],
                        start=(cb == 0), stop=(cb == NCB - 1),
                    )
                nc.vector.tensor_copy(
                    vt[:, tb, :, 0:HD],
                    ps[:, 0:256].rearrange("p (h d) -> p h d", h=4))

            def head_cfgs():
                cfgs = []
                for h in range(2):   # short heads
                    cfgs.append(dict(
                        kt=lambda kb, h=h: kts[32 * h: 32 * h + 32, kb * 128:(kb + 1) * 128],
                        qt=lambda qlo, w, h=h: qts[32 * h: 32 * h + 32, qlo: qlo + w],
                        vi=h, win=WIN_S, scale=scale_s,
                    ))
                for h in range(2):   # long heads
                    cfgs.append(dict(
                        kt=lambda kb, h=h: ktl[h][:, kb * 128:(kb + 1) * 128],
                        qt=lambda qlo, w, h=h: qtl[h][:, qlo: qlo + w],
                        vi=2 + h, win=WIN_L, scale=scale_l,
                    ))
                return cfgs

            cfgs = head_cfgs()

            def emit_score_group(qg, hi, gi, placed, mcols, pt_windows):
                """Score matmuls + exp + band mask for one score group;
                appends the pt windows to pt_windows."""
                cfg = cfgs[hi]
                q0 = qg * 512
                goffs = _GOFFS[(q0, cfg["win"])]
                if True:
                    used = placed[-1][4] + placed[-1][2]
                    st = scps.tile([128, 1024], F32, tag="scps", name="scps")
                    for kb, qlo, w, masked, off in placed:
                        # split at the 512-col psum bank boundary
                        cuts = [0]
                        if off < 512 < off + w:
                            cuts.append(512 - off)
                        cuts.append(w)
                        for a, b in zip(cuts, cuts[1:]):
                            nc.tensor.matmul(
                                st[:, off + a: off + b], cfg["kt"](kb),
                                cfg["qt"](qlo + a, b - a),
                                start=True, stop=True)
                    pt = ptp.tile([128, 1024], BF16, tag="pt", name="pt")
                    nc.scalar.activation(
                        pt[:, 0:used], st[:, 0:used],
                        mybir.ActivationFunctionType.Exp, scale=cfg["scale"])
                    if mcols:
                        # one multiply over the group's masked prefix; all on
                        # gpsimd (keeps the DVE free for psum drains) — the
                        # AV that consumes it runs a full body later, so mask
                        # latency is off the critical path
                        nc.gpsimd.tensor_tensor(
                            out=pt[:, 0:mcols], in0=pt[:, 0:mcols],
                            in1=gmask[:, goffs[gi]: goffs[gi] + mcols],
                            op=mybir.AluOpType.mult)
                    for kb, qlo, w, masked, off in placed:
                        pt_windows.append((kb, qlo, w, pt, off))

            def emit_av(qg, hi, pt_windows, yv2):
                """AV accumulation + per-head sums reciprocal + yv copy."""
                cfg = cfgs[hi]
                q0 = qg * 512
                yh = yhps.tile([VW, 512], F32, tag="yh", name="yh")
                n = len(pt_windows)
                for i, (kb, qlo, w, pt, off) in enumerate(pt_windows):
                    nc.tensor.matmul(
                        yh[:, qlo - q0: qlo - q0 + w],
                        vt[:, kb, cfg["vi"], :],
                        pt[:, off: off + w],
                        start=(i == 0), stop=(i == n - 1),
                        skip_group_check=True)
                # sums row drains on Scalar (Ln) while yv drains on DVE, so
                # the yh psum slot recycles at the speed of one copy, not two
                nc.scalar.activation(lnt[32 * hi: 32 * hi + 1, :], yh[HD: HD + 1, :],
                                     mybir.ActivationFunctionType.Ln)
                poff = 64 * (hi % 2)
                nc.vector.tensor_copy(yv2[hi // 2][poff: poff + 64, :], yh[0:HD, :])

            def emit_norm(yts, yv2, halves=(0, 1)):
                """Broadcast reciprocals via K=33 indicator matmuls and
                normalize into the bf16 yts tiles (mul reads rb psum)."""
                for half in halves:
                    rb = pops.tile([128, 512], F32, tag="pops", name="pops")
                    nc.tensor.matmul(rb[:], ind2[64 * half: 64 * half + 33, :],
                                     r4[64 * half: 64 * half + 33, :],
                                     start=True, stop=True)
                    nc.vector.tensor_mul(yts[half][:], yv2[half][:], rb[:])

            def emit_outproj(qg, yts, tail=False, subs=(0, 1, 2, 3)):
                q0 = qg * 512
                for sub in subs:
                    qs = q0 + sub * 128
                    ssl = (slice(None), slice(sub * 128, (sub + 1) * 128))
                    ob = obp.tile([128, 1024], BF16, tag="ob", name="ob")
                    for nh in range(2):
                        # po tiles live in the score-group pool: its slots are
                        # idle here, and pops stays free so the next body's
                        # projection jobs never wait on ob drains
                        po2 = scps.tile([128, 1024], F32, tag="scps", name="scps")
                        po = po2[:, 0:512]
                        nc.tensor.matmul(po[:], yts[0][ssl], wp0[:, nh * 512:(nh + 1) * 512],
                                         start=True, stop=False)
                        nc.tensor.matmul(po[:], yts[1][ssl], wp1[:, nh * 512:(nh + 1) * 512],
                                         start=False, stop=True)
                        osl = (slice(None), slice(nh * 512, (nh + 1) * 512))
                        # split psum drains between scalar and DVE
                        if nh == 1:
                            nc.vector.tensor_copy(ob[osl], po[:])
                        else:
                            nc.scalar.copy(ob[osl], po[:])
                    if tail:
                        # a [128,1024] out DMA is ~256KB on one queue (~11us);
                        # at the kernel tail nothing hides it — split across
                        # two queues
                        nc.sync.dma_start(out_d[qs: qs + 64, :], ob[0:64, :])
                        nc.sync.dma_start(out_d[qs + 64: qs + 128, :], ob[64:128, :])
                    else:
                        nc.sync.dma_start(out_d[qs: qs + 128, :], ob[:])

            # ---- merged pipeline, software-pipelined one group deep ----
            # Body t: chunk-t projections + group-t scores feed the scalar
            # exp stream, while group t-1's AV (whose exp/mask deps got a
            # full body of slack) and group t-2's output projection fill the
            # PE between score groups. The PE executes in order, so AV must
            # never sit at the end of the body that produced its deps — it
            # would bubble the PE on the exp+mask chain.
            av_pend = None    # (qg, all_pt, yv2, yts) awaiting AV in body qg+1
            norm_pend = None  # (qg, yts) normalized, awaiting outproj
            for t in range(NG):
                qg = t
                q0 = qg * 512
                yts = [ytp.tile([128, 512], BF16, tag=f"yts{i}", name=f"yts{i}")
                       for i in range(2)]
                yv2 = [smallp.tile([128, 512], BF16, tag=f"yv2{i}", name=f"yv2{i}")
                       for i in range(2)]

                # per-head score-group work queues
                sq = []
                for hi in range(4):
                    win = cfgs[hi]["win"]
                    sq.append([(hi, gi, placed, mcols)
                               for gi, (placed, mcols) in enumerate(_groups(q0, win))])
                all_pt = [[] for _ in range(4)]

                def pop_groups(heads, n):
                    done = 0
                    for hi in heads:
                        while sq[hi] and done < n:
                            _, gi, placed, mcols = sq[hi].pop(0)
                            emit_score_group(qg, hi, gi, placed, mcols, all_pt[hi])
                            done += 1
                        if done >= n:
                            break

                def pop_av(hi):
                    if av_pend is not None:
                        emit_av(av_pend[0], hi, av_pend[1][hi], av_pend[2])

                def pop_outproj(subs):
                    if norm_pend is not None:
                        emit_outproj(norm_pend[0], norm_pend[1], subs=subs)

                def emit_r4exp():
                    nc.scalar.activation(r4[:], lnt[:],
                                         mybir.ActivationFunctionType.Exp, scale=-1.0)

                if t == 0:
                    # body 0: dependency ladder (heads 0,1 need job 0; head
                    # 2 jobs 1,2; head 3 jobs 3,4)
                    emit_proj_job(0, 0)
                    pop_groups([0], 2)
                    emit_proj_job(0, 1)
                    pop_groups([0, 1], 2)
                    emit_proj_job(0, 2)
                    pop_groups([1, 2], 2)
                    emit_proj_job(0, 3)
                    pop_groups([2], 2)
                    emit_proj_job(0, 4)
                    pop_groups([2], 2)
                    emit_vproj(0, 0)
                    pop_groups([2, 3], 2)
                    emit_vproj(0, 1)
                    pop_groups([3], 2)
                    emit_vproj(0, 2)
                    pop_groups([3], 2)
                    emit_vproj(0, 3)
                    pop_groups([0, 1, 2, 3], 99)
                else:
                    # steady state: AV deps are a body old — spread AVs and
                    # outproj through the body as ready PE work
                    emit_proj_job(t, 0)
                    pop_av(0)
                    pop_groups([0], 2)
                    pop_outproj((0, 1))
                    emit_proj_job(t, 1)
                    pop_av(1)
                    pop_groups([0, 1], 2)
                    emit_proj_job(t, 2)
                    pop_groups([1, 2], 2)
                    pop_outproj((2, 3))
                    emit_proj_job(t, 3)
                    pop_av(2)
                    pop_groups([2], 2)
                    emit_proj_job(t, 4)
                    pop_groups([2], 2)
                    emit_vproj(t, 0)
                    pop_av(3)
                    emit_r4exp()
                    pop_groups([2, 3], 2)
                    emit_vproj(t, 1)
                    pop_groups([3], 2)
                    emit_vproj(t, 2)
                    pop_groups([3], 2)
                    emit_vproj(t, 3)
                    pop_groups([0, 1, 2, 3], 99)
                    if t == NG - 1:
                        # pull the short heads' AV of the last group into the
                        # body so the kernel tail only carries the long half
                        emit_av(qg, 0, all_pt[0], yv2)
                        emit_av(qg, 1, all_pt[1], yv2)
                if av_pend is not None:
                    emit_norm(av_pend[3], av_pend[2])
                    norm_pend = (av_pend[0], av_pend[3])
                av_pend = (qg, all_pt, yv2, yts)

            # ---- tail: AV g3 with half-split normalization; outproj g2
            # interleaved as PE filler, outproj g3 last
            qg, all_pt, yv2, yts = av_pend
            pop_sub = norm_pend
            nc.scalar.activation(r4[0:33, :], lnt[0:33, :],
                                 mybir.ActivationFunctionType.Exp, scale=-1.0)
            emit_outproj(pop_sub[0], pop_sub[1], subs=(0, 1), tail=True)
            emit_av(qg, 2, all_pt[2], yv2)
            emit_norm(yts, yv2, halves=(0,))
            emit_outproj(pop_sub[0], pop_sub[1], subs=(2, 3), tail=True)
            emit_av(qg, 3, all_pt[3], yv2)
            nc.scalar.activation(r4[64:97, :], lnt[64:97, :],
                                 mybir.ActivationFunctionType.Exp, scale=-1.0)
            emit_norm(yts, yv2, halves=(1,))
            emit_outproj(qg, yts, tail=True)

    return nc


_PROGRAM = None


def _get_program() -> bass.Bass:
    global _PROGRAM
    if _PROGRAM is None:
        _PROGRAM = _build_program()
        _split_waits(_PROGRAM)
    return _PROGRAM


def _gmask_image() -> np.ndarray:
    """[128, _GMASK_COLS] 0/1 image: the masked-prefix windows of every score
    group, concatenated in _gmask_layout order. Window (kb, qlo, w) column u
    covers query qlo+u against key 128*kb+r."""
    img = np.zeros((128, _GMASK_COLS), dtype=np.float32)
    r = np.arange(128)[:, None]
    col = 0
    for qg in range(NG):
        for win in (WIN_S, WIN_L):
            for placed, mcols in _groups(qg * 512, win):
                for kb, qlo, w, masked, off in placed:
                    if not masked:
                        continue
                    u = np.arange(w)[None, :]
                    d = (qlo + u) - (kb * 128 + r)
                    img[:, col: col + w] = (d >= 0) & (d < win)
                    col += w
    assert col == _GMASK_COLS
    return img


def make_in_maps(x, Wqk_short, Wv_short, Wqk_long, Wv_long, Wproj):
    """Host-side sharding: per-core input dict for core c = 4*b + g."""
    x = np.asarray(x, dtype=np.float32)
    Wqk_short = np.asarray(Wqk_short, dtype=np.float32)
    Wv_short = np.asarray(Wv_short, dtype=np.float32)
    Wqk_long = np.asarray(Wqk_long, dtype=np.float32)
    Wv_long = np.asarray(Wv_long, dtype=np.float32)
    Wproj = np.asarray(Wproj, dtype=np.float32)
    assert x.shape == (B, T, C)

    bf = bfloat16
    f8 = float8_e4m3

    def shuf_w(w, dtype, scale=1.0):
        """[C, D] -> [128, NCB*D]: row p holds c-blocks cb at stride D."""
        d = w.shape[1]
        return np.ascontiguousarray(
            (w * scale).reshape(NCB, 128, d).transpose(1, 0, 2)
            .reshape(128, NCB * d)).astype(dtype)

    xts, xbs = [], []
    for b in range(B):
        # [C, T] -> [128, tch, cb, 512]
        xtb = x[b].T.reshape(NCB, 128, NG, 512).transpose(1, 2, 0, 3)
        xtb = np.ascontiguousarray(xtb.reshape(128, NG * NCB * 512))
        xts.append(xtb.astype(f8))
        xbs.append(xtb.astype(bf))
    gmask = _gmask_image().astype(bf)
    ind2 = np.zeros((97, 128), dtype=np.float32)  # cast to bf16 below
    ind2[0, 0:64] = 1.0
    ind2[32, 64:128] = 1.0
    ind2[64, 0:64] = 1.0
    ind2[96, 64:128] = 1.0
    in_maps = []
    for c in range(N_CORES):
        b, g = divmod(c, 4)
        wsqk = shuf_w(np.concatenate(
            [Wqk_short[:, g * 64:(g + 1) * 64],
             Wqk_short[:, 256 + g * 64: 256 + (g + 1) * 64]], axis=1), f8, WSCALE)
        wql = shuf_w(Wqk_long[:, g * 256:(g + 1) * 256], f8, WSCALE)
        wkl = shuf_w(Wqk_long[:, 1024 + g * 256: 1024 + (g + 1) * 256], f8, WSCALE)
        wv = shuf_w(np.concatenate(
            [Wv_short[:, g * 128:(g + 1) * 128],
             Wv_long[:, g * 128:(g + 1) * 128]], axis=1), bf)
        wp = np.ascontiguousarray(np.concatenate(
            [Wproj[g * 128:(g + 1) * 128, :],
             Wproj[512 + g * 128: 512 + (g + 1) * 128, :]], axis=0)).astype(bf)
        in_maps.append({
            "xt": xts[b], "xb": xbs[b], "wsqk": wsqk, "wql": wql, "wkl": wkl,
            "wv": wv, "wp": wp, "gmask": gmask, "ind2": ind2.astype(bf),
        })
    return in_maps


def gather(results) -> np.ndarray:
    out = np.empty((B, T, C), dtype=np.float32)
    for b in range(B):
        acc = np.zeros((T, C), dtype=np.float32)
        for g in range(4):
            acc += results[4 * b + g]["out"].astype(np.float32)
        out[b] = acc
    return out


def kernel(x, Wqk_short, Wv_short, Wqk_long, Wv_long, Wproj, **run_kwargs):
    nc = _get_program()
    in_maps = make_in_maps(x, Wqk_short, Wv_short, Wqk_long, Wv_long, Wproj)
    res = run_bass_kernel_spmd(nc, in_maps, core_ids=list(range(N_CORES)), **run_kwargs)
    out = gather(res.results)
    if run_kwargs:
        kernel.last_results = res
    return out

